# revision 10
# baseline (speedup 1.0000x reference)
"""CapsLayer kernel v4: contraction-sharded fp8 matmul, squash on host.

Math: the reference's routing loop is dead (softmax over a size-1 axis is
identically 1), so the output is
    s[b, j, l] = sum_{i,k} W[i, j, l, k] * inputs[b, i, k]
    vj = squash(s, axis=l)  ->  [B, 1, NUM_CAPS, DIM_CAPS]

Sharding: the contraction (i, k) splits over the 8 cores (4096 of 32768
rows each), so W -- the only big tensor -- is read exactly once across the
machine and x is sliced, not replicated.  Each core emits its partial
s[b, 1024]; the host sums the 8 partials and runs the (tiny) squash.

Dtypes: W is quantized host-side to fp8 e3m4 at scale 43 (uses the top
e3m4 binade; W's native range sits in e3m4's subnormals).  x is bf16.
Measured end-to-end rel err 1.1e-2 vs the 2e-2 gate.  Per-core HBM
traffic is 4.19 MB W + 0.26 MB x + 0.13 MB out -- ~4.6 MB vs 21.2 MB for
the fp32 j-sharded v3.

PE: per 128-row tile t and 128-col block j, stationary lhsT = W tile
[128, 128] fp8, moving rhs = x tile [128, 32] bf16, accumulating into
psum group j (cols [32j, 32j+32)) over all 32 tiles.  FWL keeps weight
loads off the critical path; PE is far from the bottleneck.

DMA: W is chunked; chunks alternate between the sync and scalar engine
issue queues (the two HWDGE rings, qSPDynamicHW / qActDynamicHW) so both
rings stream concurrently.  Raw Bass: standalone wait_ge only (this
walrus build rejects multi-sem-wait instructions).
"""

from contextlib import ExitStack

import numpy as np

B = 32
IN_CAPS = 2048
IN_DIM = 16
NUM_CAPS = 32
DIM_CAPS = 32
NCORES = 8
NJL = NUM_CAPS * DIM_CAPS         # 1024 output columns (all on every core)
P = 128
IK = IN_CAPS * IN_DIM             # 32768 contraction rows total
IKC = IK // NCORES                # 4096 per core
NTILES = IKC // P                 # 32 tiles per core
NJB = NJL // P                    # 8 column blocks of 128
# chunk sizes in tiles, in PE consumption order; chunks alternate
# sync-ring (even, shares the ring with x) / scalar-ring (odd).  Tapered:
# big chunks while the pipe fills, 1-tile last chunks so the PE tail and
# final completion-semaphore lag are minimal.  Ring bytes balance:
# sync = x (2 tiles worth) + 15 tiles, scalar = 17 tiles.
CHUNKS = [5, 6, 5, 6, 4, 4, 1, 1]
NCHUNKS = len(CHUNKS)
CHUNK0 = np.cumsum([0] + CHUNKS)  # start tile of each chunk
WSCALE = np.float32(43.0)         # fp8 e3m4 scale (max |W|*43 = 15.2 < 15.5)

_CACHE = {}


def _build():
    import concourse.bass as bass
    from concourse import mybir

    f32 = mybir.dt.float32
    bf16 = mybir.dt.bfloat16
    f8 = mybir.dt.float8e3
    nc = bass.Bass()
    x = nc.declare_dram_parameter("x", [P, NTILES * B], bf16, isOutput=False)
    w = nc.declare_dram_parameter("w", [P, NTILES * NJL], f8, isOutput=False)
    out = nc.declare_dram_parameter("out", [P, NJB * B], f32, isOutput=True)

    with ExitStack() as ctx:
        x_sb = ctx.enter_context(nc.sbuf_tensor([P, NTILES * B], bf16))
        w_sb = ctx.enter_context(nc.sbuf_tensor([P, NTILES * NJL], f8))
        o_sb = ctx.enter_context(nc.sbuf_tensor([P, NJB * B], f32))
        # one accumulation group per 512-col PSUM bank: a group's start=True
        # clears has_written BANK-wide, so groups must not share a bank
        ps = ctx.enter_context(nc.psum_tensor([P, NJB * 512], f32))

        xs = ctx.enter_context(nc.semaphore("xs"))
        wsem = [ctx.enter_context(nc.semaphore(f"w{c}")) for c in range(NCHUNKS)]
        pe_sem = ctx.enter_context(nc.semaphore("pe"))
        cp_sem = ctx.enter_context(nc.semaphore("cp"))
        odma = ctx.enter_context(nc.semaphore("odma"))
        block = ctx.enter_context(nc.Block())

        def wcols(c):
            return slice(CHUNK0[c] * NJL, CHUNK0[c + 1] * NJL)

        @block.sync
        def _(sync):
            sync.dma_start(out=x_sb[:, :], in_=x[:, :]).then_inc(xs, 16)
            for c in range(0, NCHUNKS, 2):
                sync.dma_start(
                    out=w_sb[:, wcols(c)], in_=w[:, wcols(c)]
                ).then_inc(wsem[c], 16)
            sync.wait_ge(cp_sem, 1)
            sync.dma_start(out=out[:, :], in_=o_sb[:, :]).then_inc(odma, 16)
            sync.wait_ge(odma, 16)

        @block.scalar
        def _(scalar):
            for c in range(1, NCHUNKS, 2):
                scalar.dma_start(
                    out=w_sb[:, wcols(c)], in_=w[:, wcols(c)]
                ).then_inc(wsem[c], 16)

        @block.vector
        def _(vector):
            vector.wait_ge(pe_sem, 1)
            psv = ps[:, :].rearrange("p (j c) -> p j c", c=512)[:, :, 0:B]
            nc.vector.tensor_copy(o_sb[:, :], psv).then_inc(cp_sem, 1)

        @block.tensor
        def _(tensor):
            tensor.wait_ge(xs, 16)
            for c in range(NCHUNKS):
                tensor.wait_ge(wsem[c], 16)
                for t in range(CHUNK0[c], CHUNK0[c + 1]):
                    for j in range(NJB):
                        mm = nc.tensor.matmul(
                            ps[:, 512 * j:512 * j + B],
                            w_sb[:, t * NJL + P * j:t * NJL + P * (j + 1)],
                            x_sb[:, t * B:(t + 1) * B],
                            start=(t == 0),
                            stop=(t == NTILES - 1),
                        )
            mm.then_inc(pe_sem, 1)

    return nc


def _in_maps(inputs, W):
    import ml_dtypes

    f8 = ml_dtypes.float8_e3m4
    bf16 = ml_dtypes.bfloat16
    # [(i,k), (j,l)] / [(i,k), b] contraction-major flats
    w_t = W.transpose(0, 3, 1, 2).reshape(IK, NJL)
    x_t = inputs.transpose(1, 2, 0).reshape(IK, B)
    maps = []
    for c in range(NCORES):
        ik0 = c * IKC
        wc = (w_t[ik0:ik0 + IKC] * WSCALE).astype(f8)
        xc = x_t[ik0:ik0 + IKC].astype(bf16)
        maps.append({
            "w": np.ascontiguousarray(
                wc.reshape(NTILES, P, NJL).transpose(1, 0, 2)
            ).reshape(P, NTILES * NJL),
            "x": np.ascontiguousarray(
                xc.reshape(NTILES, P, B).transpose(1, 0, 2)
            ).reshape(P, NTILES * B),
        })
    return maps


def kernel(inputs, W):
    from concourse.bass_utils import run_bass_kernel_spmd

    inputs = np.asarray(inputs, dtype=np.float32)
    W = np.asarray(W, dtype=np.float32)
    if "nc" not in _CACHE:
        _CACHE["nc"] = _build()
    res = run_bass_kernel_spmd(_CACHE["nc"], _in_maps(inputs, W), list(range(NCORES)))
    # out[p, B*j + b] = s_c[b, 128*j + p]; sum partials over cores
    s = np.zeros((B, NJL), dtype=np.float32)
    for c in range(NCORES):
        o = np.asarray(res.results[c]["out"], dtype=np.float32)
        s += o.reshape(P, NJB, B).transpose(2, 1, 0).reshape(B, NJL)
    s = (s / WSCALE).reshape(B, NUM_CAPS, DIM_CAPS)
    ss = np.sum(s * s, axis=-1, keepdims=True)
    vj = (ss / (1.0 + ss)) * (s / np.sqrt(ss + 1e-7))
    return vj[:, None, :, :].astype(np.float32)
